# revision 89
# baseline (speedup 1.0000x reference)
"""Trainium2 Bass kernel for nn_Block (dense transformer block).

  out = x + FFN(LN2(x + Attn(LN1(x))))   with causal single-head attention,
  B=4, T=2048, C=H=1024, FF=4096, fp32 reference.

Distribution: 8 NeuronCores = (batch b in 0..3) x (query-half in 0..1).
Each core handles one batch element's keys/values and HALF its query rows
(causally balanced interleaved block split), plus LN2+FFN+residual for those
rows.  No collectives; the per-core programs are IDENTICAL (SPMD) - all
per-core variation is input data.

v4 (from v3 @398us):
 - v projection moved to fp8 DoubleRow (frees 128 bf16 PE slots); LN1 h^T
   is written fp8 straight from PSUM by the Scalar engine (kills the slow
   GpSimd bf16->fp8 CASTs and the trb intermediate on the b-path).
 - startup: identity DMA first on the otherwise-idle Scalar hwdge queue;
   qb/kb/b1 are host-prepped [P,m]-contiguous (the old "(m p)->p m"
   4-byte-element gather DMA took 9.9us and blocked the sync queue).
 - LN rstd = Rsqrt activation (one Scalar op, no Vector reciprocal).
 - own-token LN (a-path) spread over st 0..7 (Vector slack), k^T built in
   single 512-key chunks right after each h^T half lands (st 4/8/12/16).
 - engine rebalance: a/h2 LN normalize on GpSimd, xbar-path fp8 casts on
   Vector, one v-copy each on Scalar/GpSimd.
 - end-of-attention: h2 xbar transposes for blocks 4,5 issued before the
   last two tails; FFN1 token-half 0 for ft 0..15 hoisted between tails 6/7
   and the PE-path h2 transposes (blocks 6,7 write h2T fp8 direct from
   PSUM), so the PE stays fed while Vector drains the tail chains.
 - FFN1 runs tch0 for all ft, then tch1 (w1 streamed twice); FFN2 for
   blocks 0..3 interleaves into the tch1 stream.
Accumulation is always f32 in PSUM; LN stats / softmax / residuals f32;
softmax-weights @ v stays bf16.
"""

import sys
import types

import numpy as np

# ---------------------------------------------------------------------------
# antenv.axon_hooks shim: the image's antenv lacks this module and
# run_bass_kernel_spmd imports it under axon when trace=True.
import antenv

if "antenv.axon_hooks" not in sys.modules:
    _mod = types.ModuleType("antenv.axon_hooks")
    _mod._hook = None
    _mod.set_axon_ntff_profile_hook = lambda h: setattr(_mod, "_hook", h)
    _mod.get_axon_ntff_profile_hook = lambda: _mod._hook
    sys.modules["antenv.axon_hooks"] = _mod
    antenv.axon_hooks = _mod

import ml_dtypes

import concourse.bass as bass
import concourse.mybir as mybir
import concourse.tile as tile
from concourse.bass_utils import run_bass_kernel_spmd

F32 = mybir.dt.float32
BF16 = mybir.dt.bfloat16
F8 = mybir.dt.float8e4
DR = mybir.MatmulPerfMode.DoubleRow
AF = mybir.ActivationFunctionType
ALU = mybir.AluOpType

B, T, C = 4, 2048, 1024
H, FF = 1024, 4096
P = 128
NT = T // P  # 16 token blocks per batch element
NCT = C // P  # 8 contraction tiles
NH = H // P  # 8 head-dim tiles
NF = FF // P  # 32 ff tiles
TOWN = T // 2  # own tokens per core (1024)
NLOC = TOWN // P  # 8 own blocks
EPS = 1e-5
SW = 32.0  # fp8 weight pre-scale
EXP_SCALE = (1.0 / np.sqrt(np.float32(C))) / (SW * SW)  # folded softmax scale
VINV = 1.0 / SW  # v psum unscale (single SW factor)
FF2_INV = 1.0 / (SW * SW)  # FFN2 psum unscale
NEG = -1.0e30

# Causally balanced query-block assignment (sum of chunk counts = 20 each).
L_HALF = [
    [0, 2, 4, 6, 9, 11, 13, 15],
    [1, 3, 5, 7, 8, 10, 12, 14],
]
# ceil((i+1)/4) for i in L_HALF[h] - same sequence for both halves.
NCHUNKS = [1, 1, 2, 2, 3, 3, 4, 4]
# Per-half KEY-TILE permutation (position -> global tile).  Chosen so that
# each core's OWN blocks sit at the SAME positions OWN_POS in its permuted
# key space (program stays SPMD-uniform; masks carry the causality), and
# all non-causal tiles land in each query block's LAST 512-key chunk.
PERMS = [
    list(range(NT)),
    [1, 0, 3, 2, 5, 4, 7, 6, 9, 8, 11, 10, 13, 12, 15, 14],
]
OWN_POS = L_HALF[0]  # own blocks' permuted positions, both halves


def _split_multi_waits(nc):
    """walrus here accepts at most ONE sync-wait per instruction; hoist
    extras onto injected same-engine NoOps."""
    for fn in nc.m.functions:
        for blk in fn.blocks:
            new_insts = []
            changed = False
            for inst in blk.instructions:
                si = getattr(inst, "sync_info", None)
                ow = list(si.on_wait) if si is not None and si.on_wait else []
                if len(ow) > 1:
                    for i, cond in enumerate(ow[:-1]):
                        new_insts.append(
                            mybir.InstNoOp(
                                name=f"{inst.name}-wn{i}",
                                engine=inst.engine,
                                ins=[],
                                outs=[],
                                sync_info=mybir.SyncInfo(
                                    on_wait=[cond], on_update=[]
                                ),
                            )
                        )
                    inst.sync_info = mybir.SyncInfo(
                        on_wait=[ow[-1]], on_update=list(si.on_update or [])
                    )
                    changed = True
                new_insts.append(inst)
            if changed:
                blk.instructions = new_insts


def build_nc():
    from contextlib import ExitStack

    nc = bass.Bass()

    x = nc.declare_dram_parameter("x", [T, C], F32, isOutput=False)
    x_own = nc.declare_dram_parameter("x_own", [TOWN, C], F32, isOutput=False)
    wq = nc.declare_dram_parameter("wq", [P, NCT, H], F8, isOutput=False)
    wk = nc.declare_dram_parameter("wk", [P, NCT, H], F8, isOutput=False)
    wv = nc.declare_dram_parameter("wv", [P, NCT, H], F8, isOutput=False)
    w1 = nc.declare_dram_parameter("w1", [P, NF, NCT, P], F8, isOutput=False)
    w2 = nc.declare_dram_parameter("w2", [P, NF, C], F8, isOutput=False)
    # qb | kb | b1 packed into one [P, 48] tensor (single fast DMA)
    cb = nc.declare_dram_parameter("cb", [P, 2 * NH + NF], F32, isOutput=False)
    vb = nc.declare_dram_parameter("vb", [H], F32, isOutput=False)  # unscaled
    b2 = nc.declare_dram_parameter("b2", [C], F32, isOutput=False)
    masks = nc.declare_dram_parameter("masks", [NLOC, P, 512], BF16, isOutput=False)
    out = nc.declare_dram_parameter("out", [TOWN, C], F32, isOutput=True)

    with tile.TileContext(nc) as tc, ExitStack() as top:
        cn = top.enter_context(tc.tile_pool(name="cn", bufs=1))
        ps = top.enter_context(tc.tile_pool(name="ps", bufs=1, space="PSUM"))
        ln = top.enter_context(tc.tile_pool(name="ln", bufs=1))

        # identity built on-device (any DMA of its 256B rows costs 6-10us):
        # iota(f - p) == 0
        id16 = cn.tile([P, P], mybir.dt.int16)
        nc.gpsimd.iota(id16[:], [[1, P]], channel_multiplier=-1)
        id_t = cn.tile([P, P], BF16)
        nc.gpsimd.tensor_scalar(
            out=id_t[:], in0=id16[:], scalar1=0, scalar2=None,
            op0=ALU.is_equal,
        )
        # critical path: x tile 0 for LN1 on the (startup-idle) scalar ring,
        # in parallel with x1.. on the sync ring.
        x0_t = ln.tile([P, C], F32, tag="xt", bufs=3, name="x0t")
        nc.scalar.dma_start(x0_t[:], x[0:P, :])
        # packed small constants: one DMA, on the GpSimd ring (its 192B
        # packets would stall the sync ring's x-tile stream)
        cb_t = cn.tile([P, 2 * NH + NF], F32)
        nc.gpsimd.dma_start(cb_t[:], cb[:])
        # all causal masks in one DMA ([NLOC,P,512] -> [P,NLOC,512]);
        # issued on the GpSimd ring so the sync queue keeps feeding x tiles
        mask_all = cn.tile([P, NLOC, 512], BF16)
        qb_t = cb_t[:, 0:NH]
        kb_t = cb_t[:, NH : 2 * NH]
        b1_t = cb_t[:, 2 * NH : 2 * NH + NF]
        eps_t = cn.tile([P, 1], F32)
        nc.vector.memset(eps_t, EPS)
        # activation-table warmup: touch every (func, out-dtype) combo used
        # later so the 1.3us ACT_TABLE_LOADs happen while DMAs stream.
        wrm = cn.tile([P, 8], F32)
        wrm8 = cn.tile([P, 8], F8)
        wrmb = cn.tile([P, 8], BF16)
        nc.scalar.activation(out=wrm[:, 0:1], in_=eps_t[:], func=AF.Sqrt,
                             bias=0.0, scale=1.0)
        nc.scalar.activation(out=wrm8[:, 0:1], in_=eps_t[:], func=AF.Identity,
                             bias=0.0, scale=1.0)
        nc.scalar.activation(out=wrmb[:, 0:1], in_=eps_t[:], func=AF.Identity,
                             bias=0.0, scale=1.0)
        nc.scalar.activation(out=wrmb[:, 1:2], in_=eps_t[:], func=AF.Exp,
                             bias=0.0, scale=1.0)
        nc.scalar.activation(out=wrm8[:, 1:2], in_=eps_t[:], func=AF.Relu,
                             bias=0.0, scale=1.0)
        nc.scalar.activation(out=wrm8[:, 2:3], in_=wrmb[:, 0:1],
                             func=AF.Identity, bias=0.0, scale=1.0)

        px = top.enter_context(tc.tile_pool(name="px", bufs=1))
        x2s = px.tile([P, NLOC, C], F32)    # attn residual, SBUF-resident
        h2T = px.tile([P, NCT, TOWN], F8)   # LN2 output transposed
        w1p = px.tile([P, 4, NCT, P], F8)   # prefetched w1 ft 0..3
        w1p2 = px.tile([P, 4, NCT, P], F8)  # prefetched w1 ft 4..7
        aTh = px.tile([P, 8, 512], F8)      # hoisted FFN1 tch0, ft 0..7

        _ctr = [0]

        def psum(tag, shape=(P, 512), dt=F32, bufs=2):
            _ctr[0] += 1
            return ps.tile(list(shape), dt, tag=tag, bufs=bufs, name=f"ps{_ctr[0]}")

        def layernorm(x_ap, h_t, norm=None):
            """h_t (bf16) = (x - mean) * rsqrt(var + eps), stats on free dim.
            norm: engine for the normalize pass (default Vector)."""
            xg = x_ap.rearrange("p (s f) -> p s f", f=512)
            stats = ln.tile([P, 2, nc.vector.BN_STATS_DIM], F32,
                            tag="ln_stats", bufs=3)
            for sg in range(2):
                nc.vector.bn_stats(out=stats[:, sg], in_=xg[:, sg])
            mv = ln.tile([P, nc.vector.BN_AGGR_DIM], F32, tag="ln_mv", bufs=3)
            nc.vector.bn_aggr(out=mv[:], in_=stats[:])
            rstd = ln.tile([P, 1], F32, tag="ln_rstd", bufs=3)
            nc.scalar.activation(
                out=rstd[:], in_=mv[:, 1:2], func=AF.Sqrt,
                bias=eps_t[:], scale=1.0,
            )
            nc.vector.reciprocal(out=rstd[:], in_=rstd[:])
            if norm == "scalar":
                # normalize on the Scalar engine: x*rstd + (-mu*rstd).
                # Used for the h2 path, where Vector is the serial tail
                # chain and Scalar sits idle.
                nmr = ln.tile([P, 1], F32, tag="ln_nmr", bufs=3)
                nc.vector.scalar_tensor_tensor(
                    out=nmr[:], in0=mv[:, 0:1], scalar=-1.0, in1=rstd[:],
                    op0=ALU.mult, op1=ALU.mult,
                )
                nc.scalar.activation(
                    out=h_t[:], in_=x_ap, func=AF.Identity,
                    bias=nmr[:], scale=rstd[:],
                )
            else:
                nc.vector.tensor_scalar(
                    out=h_t[:], in0=x_ap,
                    scalar1=mv[:, 0:1], scalar2=rstd[:],
                    op0=ALU.subtract, op1=ALU.mult,
                )

        def ln_front(x_ap, tag="ht", bufs=3, norm=None):
            h_t = ln.tile([P, C], BF16, tag=tag, bufs=bufs, name=tag)
            layernorm(x_ap, h_t, norm=norm)
            return h_t

        def pe_transpose_to_f8(h_t, dst_ap):
            """h_t [P,C] bf16 -> PE transposes -> Scalar writes fp8 dst
            [P,NCT,P] straight from PSUM (no bf16 intermediate)."""
            tp = psum("t", (P, 8 * P), BF16)
            for c in range(NCT):
                nc.tensor.transpose(
                    tp[:, c * P : (c + 1) * P],
                    h_t[:, c * P : (c + 1) * P],
                    id_t[:],
                )
            nc.scalar.activation(
                out=dst_ap,
                in_=tp[:].rearrange("p (s f) -> p s f", f=P),
                func=AF.Identity, bias=0.0, scale=1.0,
            )

        with ExitStack() as sABC:
            pqv = sABC.enter_context(tc.tile_pool(name="pqv", bufs=1))
            kT = pqv.tile([P, NH, T], F8)        # k^T all keys
            v_sb = pqv.tile([P, NT, H], BF16)    # v token-major (bf16)
            att = sABC.enter_context(tc.tile_pool(name="att", bufs=2))
            wtl = sABC.enter_context(tc.tile_pool(name="wtl", bufs=2))

            pac = sABC.enter_context(tc.tile_pool(name="pac", bufs=1))
            qTo = pac.tile([P, NH, TOWN], F8)    # q^T own tokens (local order)
            hto = pac.tile([P, NCT, TOWN], F8)
            wq_s = pac.tile([P, NCT, H], F8, name="wqs")

            def q_group(g):
                # q^T for own 512-token group g; stationary reused 2x
                for m in range(NH):
                    acc = psum("c", bufs=4)
                    for k2 in range(NCT // 2):
                        nc.tensor.matmul(
                            acc[:],
                            wq_s[:, 2 * k2 : 2 * k2 + 2, m * P : (m + 1) * P],
                            hto[:, 2 * k2 : 2 * k2 + 2, g * 512 : (g + 1) * 512],
                            start=(k2 == 0),
                            stop=(k2 == NCT // 2 - 1),
                            perf_mode=DR,
                        )
                    nc.scalar.activation(
                        out=qTo[:, m, g * 512 : (g + 1) * 512],
                        in_=acc[:], func=AF.Identity,
                        bias=qb_t[:, m : m + 1], scale=1.0,
                    )

            state = {}
            h2state = {}

            def emit_scores(lp):
                nch = NCHUNKS[lp]
                # prefetch own-x for the tail on the GpSimd ring: the
                # scheduler hoists hwdge DMAs with no deps into the startup
                # window, stealing HBM from the x-tile stream
                x_t = att.tile([P, C], F32, tag="xo", bufs=2)
                nc.gpsimd.dma_start(x_t[:], x_own[lp * P : (lp + 1) * P, :])
                nc.vector.tensor_add(out=x_t[:], in0=x_t[:], in1=vb_b[:])
                p_t = att.tile([P, T], BF16, tag="pt", bufs=3)
                den = att.tile([P, 4], F32, tag="den", bufs=4)
                scs = [psum("c", bufs=4) for _ in range(nch)]
                for m2 in range(NH // 2):
                    for j in range(nch):
                        nc.tensor.matmul(
                            scs[j][:],
                            qTo[:, 2 * m2 : 2 * m2 + 2, lp * P : (lp + 1) * P],
                            kT[:, 2 * m2 : 2 * m2 + 2, j * 512 : (j + 1) * 512],
                            start=(m2 == 0),
                            stop=(m2 == NH // 2 - 1),
                            perf_mode=DR,
                        )
                for j in range(nch):
                    if j == nch - 1:
                        nc.vector.tensor_add(
                            out=scs[j][:], in0=scs[j][:], in1=mask_all[:, lp]
                        )
                    nc.scalar.activation(
                        out=p_t[:, j * 512 : (j + 1) * 512],
                        in_=scs[j][:], func=AF.Exp,
                        scale=float(EXP_SCALE),
                        accum_out=den[:, j : j + 1],
                    )
                state[lp] = (p_t, den, x_t)

            def emit_tail(lp):
                nch = NCHUNKS[lp]
                nst = 4 * nch
                p_t, den, xvb = state.pop(lp)
                dsum = att.tile([P, 1], F32, tag="dsum")
                nc.vector.reduce_sum(
                    out=dsum[:], in_=den[:, :nch], axis=mybir.AxisListType.X
                )
                nc.vector.reciprocal(out=dsum[:], in_=dsum[:])
                # p^T via PE transposes (bf16)
                wtT = wtl.tile([P, 16, P], BF16, tag="wt")
                for tg in range((nst + 3) // 4):
                    n4 = min(4, nst - tg * 4)
                    tp = psum("t", (P, 8 * P), BF16)
                    for i in range(n4):
                        stp = tg * 4 + i
                        nc.tensor.transpose(
                            tp[:, i * P : (i + 1) * P],
                            p_t[:, stp * P : (stp + 1) * P],
                            id_t[:],
                        )
                    # p^T copies: Scalar for the LAST two tails (keeps the
                    # Vector-serial end chain off the p@v critical path);
                    # Vector mid-phase, where Scalar's backlog would delay
                    # the "c"-psum drains that the score matmuls rotate on
                    if lp >= NLOC - 2:
                        nc.scalar.activation(
                            out=wtT[:, tg * 4 : tg * 4 + n4, :],
                            in_=tp[:].rearrange("p (s f) -> p s f", f=P)[:, :n4],
                            func=AF.Identity, bias=0.0, scale=1.0,
                        )
                    else:
                        nc.vector.tensor_copy(
                            out=wtT[:, tg * 4 : tg * 4 + n4, :],
                            in_=tp[:].rearrange("p (s f) -> p s f", f=P)[:, :n4],
                        )
                sa0 = psum("a", bufs=2)
                sa1 = psum("a", bufs=2)
                for stp in range(nst):
                    nc.tensor.matmul(
                        sa0[:], wtT[:, stp, :], v_sb[:, stp, 0:512],
                        start=(stp == 0), stop=(stp == nst - 1),
                    )
                    nc.tensor.matmul(
                        sa1[:], wtT[:, stp, :], v_sb[:, stp, 512:1024],
                        start=(stp == 0), stop=(stp == nst - 1),
                    )
                for cc, sa in ((0, sa0), (1, sa1)):
                    nc.vector.scalar_tensor_tensor(
                        out=x2s[:, lp, cc * 512 : (cc + 1) * 512],
                        in0=sa[:],
                        scalar=dsum[:],
                        in1=xvb[:, cc * 512 : (cc + 1) * 512],
                        op0=ALU.mult,
                        op1=ALU.add,
                    )
                # LN2 front for this block (back is staged later)
                h2state[lp] = ln_front(x2s[:, lp, :])

            def h2_back(lp, path="pe"):
                pe_transpose_to_f8(
                    h2state.pop(lp), h2T[:, :, lp * P : (lp + 1) * P]
                )

            with ExitStack() as sB:
                pab = sB.enter_context(tc.tile_pool(name="pab", bufs=1))
                # rolling h^T buffer: halves ping-pong between chunk pairs
                # (0,2) on half0 and (1,3) on half1; k consumes each chunk
                # right after its 4 token-tiles land.
                hT = pab.tile([P, NCT, 2, 512], F8)
                # projection weights on the GpSimd DMA queue in need order:
                # wv (st1), wk (st4), wq (st13); broadcasts later.
                wv_s = pab.tile([P, NCT, H], F8, name="wvs")
                nc.gpsimd.dma_start(wv_s[:], wv[:])
                wk_s = pab.tile([P, NCT, H], F8, name="wks")
                nc.gpsimd.dma_start(wk_s[:], wk[:])
                nc.gpsimd.dma_start(
                    mask_all[:], masks.rearrange("l p f -> p l f")
                )
                vb_b = cn.tile([P, H], BF16)
                b2_b = cn.tile([P, C], BF16)

                def k_chunk(ch):
                    # k^T for one 512-key chunk (stationary not reused, but
                    # the DR LDWEIGHTS hide under the 512-wide matmuls);
                    # PSUM->kT copies alternate Scalar/Vector.
                    half = ch % 2
                    for m in range(NH):
                        acc = psum("c", bufs=4)
                        for k2 in range(NCT // 2):
                            nc.tensor.matmul(
                                acc[:],
                                wk_s[:, 2 * k2 : 2 * k2 + 2, m * P : (m + 1) * P],
                                hT[:, 2 * k2 : 2 * k2 + 2, half, :],
                                start=(k2 == 0),
                                stop=(k2 == NCT // 2 - 1),
                                perf_mode=DR,
                            )
                        nc.scalar.activation(
                            out=kT[:, m, ch * 512 : (ch + 1) * 512],
                            in_=acc[:], func=AF.Identity,
                            bias=kb_t[:, m : m + 1], scale=1.0,
                        )

                def b_back(st, h_t):
                    half = (st // 4) % 2
                    # h^T fp8 written straight from transpose PSUM; when st
                    # is one of this core's OWN blocks (uniform positions by
                    # the host-side key permutation), the same PSUM also
                    # feeds the contiguous own-token h^T (hto).
                    tp = psum("t", (P, 8 * P), BF16)
                    for c in range(NCT):
                        nc.tensor.transpose(
                            tp[:, c * P : (c + 1) * P],
                            h_t[:, c * P : (c + 1) * P],
                            id_t[:],
                        )
                    tp3 = tp[:].rearrange("p (s f) -> p s f", f=P)
                    nc.scalar.activation(
                        out=hT[:, :, half, (st % 4) * P : (st % 4 + 1) * P],
                        in_=tp3, func=AF.Identity, bias=0.0, scale=1.0,
                    )
                    if st in OWN_POS:
                        # own-block h^T copied on the otherwise-idle GpSimd
                        # (SBUF->SBUF; it may not read PSUM), freeing tp
                        # after a single Scalar read
                        lt = OWN_POS.index(st)
                        nc.gpsimd.tensor_copy(
                            out=hto[:, :, lt * P : (lt + 1) * P],
                            in_=hT[:, :, half, (st % 4) * P : (st % 4 + 1) * P],
                        )
                    # v row-block from fp8 h^T (DoubleRow)
                    va = [psum("c", bufs=4) for _ in range(2)]
                    for k2 in range(NCT // 2):
                        for hh in range(2):
                            nc.tensor.matmul(
                                va[hh][:],
                                hT[:, 2 * k2 : 2 * k2 + 2, half,
                                   (st % 4) * P : (st % 4 + 1) * P],
                                wv_s[:, 2 * k2 : 2 * k2 + 2,
                                     hh * 512 : (hh + 1) * 512],
                                start=(k2 == 0),
                                stop=(k2 == NCT // 2 - 1),
                                perf_mode=DR,
                            )
                    # unscale SW -> bf16; one copy each on Scalar / Vector
                    nc.scalar.activation(
                        out=v_sb[:, st, 0:512], in_=va[0][:],
                        func=AF.Identity, bias=0.0, scale=VINV,
                    )
                    nc.vector.tensor_scalar(
                        out=v_sb[:, st, 512:1024], in0=va[1][:],
                        scalar1=VINV, scalar2=None, op0=ALU.mult,
                    )

                # ===== merged B/C-start stream, 2-stage emission ===========
                b_front = {}
                for st in range(NT + 1):
                    if st < NT:
                        if st == 0:
                            x_t = x0_t
                        else:
                            x_t = ln.tile([P, C], F32, tag="xt", bufs=3)
                            nc.sync.dma_start(
                                x_t[:], x[st * P : (st + 1) * P, :]
                            )
                        b_front[st] = ln_front(x_t[:])
                    if st >= 1:
                        b_back(st - 1, b_front.pop(st - 1))
                        if st % 4 == 0:
                            k_chunk(st // 4 - 1)
                    if st == 1:
                        nc.gpsimd.dma_start(wq_s[:], wq[:])
                    elif st == 2:
                        nc.gpsimd.dma_start(
                            vb_b[:], vb[None, :].partition_broadcast(P)
                        )
                    elif st == 6:
                        nc.gpsimd.dma_start(
                            b2_b[:], b2[None, :].partition_broadcast(P)
                        )
                    # own-token h^T (hto) comes straight out of b_back, so
                    # q/scores can start as soon as their blocks land.
                    if st == 9:
                        q_group(0)
                    elif st == 11:
                        emit_scores(0)
                    elif st == 13:
                        emit_scores(1)

            # ================= Phase C: attention pipeline =================
            q_group(1)
            # prefetch the first w1 group on the GpSimd ring (the scheduler
            # hoists sync-ring DMAs ahead of the x-tile stream otherwise)
            nc.gpsimd.dma_start(w1p[:], w1[:, 0:4])
            nc.gpsimd.dma_start(w1p2[:], w1[:, 4:8])
            for lp in range(2, NLOC):
                emit_scores(lp)
                emit_tail(lp - 2)
                if lp >= 4:
                    h2_back(lp - 4, "xbar")

            # ready h2 transposes + hoisted FFN1 (token-half 0 of the
            # prefetched ft 0..3, which needs only h2 blocks 0..3)
            # interleave as PE filler while Vector drains the tail chains
            emit_tail(NLOC - 2)
            h2_back(4)
            emit_tail(NLOC - 1)
            h2_back(5)
            for ft in range(8):
                wp = w1p if ft < 4 else w1p2
                acc = psum("c", bufs=4)
                for k2 in range(NCT // 2):
                    nc.tensor.matmul(
                        acc[:],
                        wp[:, ft % 4, 2 * k2 : 2 * k2 + 2, :],
                        h2T[:, 2 * k2 : 2 * k2 + 2, 0:512],
                        start=(k2 == 0),
                        stop=(k2 == NCT // 2 - 1),
                        perf_mode=DR,
                    )
                # bias+relu on Vector (add+max): keeps Scalar free for the
                # tail-7 p^T copies that gate the h2 transpose PSUM reuse
                nc.vector.tensor_scalar(
                    out=aTh[:, ft, :], in0=acc[:],
                    scalar1=b1_t[:, ft : ft + 1], scalar2=0.0,
                    op0=ALU.add, op1=ALU.max,
                )
                if ft == 3:
                    h2_back(6)
            h2_back(7)

        # ================= Phase D: FFN (fp8 DR) ===========================
        # tch0 for every ft first (only needs h2 blocks 0..3, which land
        # mid-attention), then tch1; FFN2 for the tch0 blocks interleaves
        # into the tch1 stream.  w1 is streamed twice (HBM is cheap here).
        with ExitStack() as sD:
            big_d = sD.enter_context(tc.tile_pool(name="bigd", bufs=1))
            ffw = sD.enter_context(tc.tile_pool(name="ffw", bufs=3))
            aT0 = big_d.tile([P, NF, 512], F8)   # a^T all ft, token half 0
            aT1 = big_d.tile([P, NF, 512], F8)   # a^T all ft, token half 1
            w2_s = big_d.tile([P, NF, C], F8, name="w2s")

            def ffn1(wt, fi, ft, tchs=(0, 1)):
                # single w1 tile feeds BOTH token halves (one HBM pass)
                wsl = lambda k2: wt[:, fi, 2 * k2 : 2 * k2 + 2, :]
                accs = {tch: psum("c", bufs=4) for tch in tchs}
                for k2 in range(NCT // 2):
                    for tch in tchs:
                        nc.tensor.matmul(
                            accs[tch][:],
                            wsl(k2),
                            h2T[:, 2 * k2 : 2 * k2 + 2,
                                tch * 512 : (tch + 1) * 512],
                            start=(k2 == 0),
                            stop=(k2 == NCT // 2 - 1),
                            perf_mode=DR,
                        )
                for tch in tchs:
                    # the two PSUM drains run on different engines so the
                    # "c"/"a" rotations clear in parallel
                    dst = aT0 if tch == 0 else aT1
                    if tch == 0:
                        nc.scalar.activation(
                            out=dst[:, ft, :], in_=accs[tch][:], func=AF.Relu,
                            bias=b1_t[:, ft : ft + 1], scale=1.0,
                        )
                    else:
                        nc.vector.tensor_scalar(
                            out=dst[:, ft, :], in0=accs[tch][:],
                            scalar1=b1_t[:, ft : ft + 1], scalar2=0.0,
                            op0=ALU.add, op1=ALU.max,
                        )

            def ffn2(lt):
                xb = ffw.tile([P, C], F32, tag="xb", name=f"xb{lt}")
                nc.vector.tensor_add(out=xb[:], in0=x2s[:, lt, :], in1=b2_b[:])
                # borrow the (phase-D-idle) "c" rotation so consecutive
                # blocks don't WAR-stall on the 2-deep "a" tag
                grp = [psum("c", bufs=4), psum("a", bufs=2)]
                for f2 in range(NF // 2):
                    if lt < 4:
                        src = aTh if f2 < 4 else aT0
                        aslc = src[:, 2 * f2 : 2 * f2 + 2,
                                   lt * P : (lt + 1) * P]
                    else:
                        aslc = aT1[:, 2 * f2 : 2 * f2 + 2,
                                   (lt - 4) * P : (lt - 3) * P]
                    for cc in range(2):
                        nc.tensor.matmul(
                            grp[cc][:],
                            aslc,
                            w2_s[:, 2 * f2 : 2 * f2 + 2,
                                 cc * 512 : (cc + 1) * 512],
                            start=(f2 == 0),
                            stop=(f2 == NF // 2 - 1),
                            perf_mode=DR,
                        )
                o_t = ffw.tile([P, C], F32, tag="ot", name=f"ot{lt}")
                for cc in range(2):
                    nc.vector.scalar_tensor_tensor(
                        out=o_t[:, cc * 512 : (cc + 1) * 512],
                        in0=grp[cc][:],
                        scalar=FF2_INV,
                        in1=xb[:, cc * 512 : (cc + 1) * 512],
                        op0=ALU.mult,
                        op1=ALU.add,
                    )
                    # store each half as soon as its residual add lands
                    nc.sync.dma_start(
                        out[lt * P : (lt + 1) * P, cc * 512 : (cc + 1) * 512],
                        o_t[:, cc * 512 : (cc + 1) * 512],
                    )

            # w2 (4MB) on the GpSimd ring - it has ~60us before FFN2 starts
            nc.gpsimd.dma_start(w2_s[:], w2[:])
            # w1 streamed in 4-ft groups (4KB contiguous per partition -
            # single-ft tiles are 1KB-packet-bound on the DMA ring)
            for fg in range(NF // 4):
                if fg == 0:
                    wt = w1p
                elif fg == 1:
                    wt = w1p2
                else:
                    wt = ffw.tile([P, 4, NCT, P], F8, tag="w1t", bufs=3,
                                  name=f"w1g{fg}")
                    nc.sync.dma_start(wt[:], w1[:, 4 * fg : 4 * fg + 4])
                for fi in range(4):
                    # ft 0..7 token-half 0 was hoisted into the attention
                    # tail (aTh); only half 1 remains for them
                    ffn1(wt, fi, 4 * fg + fi,
                         tchs=(1,) if fg < 2 else (0, 1))
            for lt in range(NLOC):
                ffn2(lt)

    _split_multi_waits(nc)
    return nc


_NC_CACHE = None


def _get_nc():
    global _NC_CACHE
    if _NC_CACHE is None:
        _NC_CACHE = build_nc()
    return _NC_CACHE


def _to_f8(a):
    return np.ascontiguousarray(
        np.clip(a, -240.0, 240.0).astype(ml_dtypes.float8_e4m3)
    )


def _prep_host(inputs):
    """Fold LN gains/biases into weights; scale weights x32 for fp8;
    build per-core input maps."""
    x = np.asarray(inputs["x"], dtype=np.float32)
    Wk = np.asarray(inputs["Wk"], dtype=np.float32)
    Wq = np.asarray(inputs["Wq"], dtype=np.float32)
    Wv = np.asarray(inputs["Wv"], dtype=np.float32)
    W1 = np.asarray(inputs["W1"], dtype=np.float32)
    b1 = np.asarray(inputs["b1"], dtype=np.float32)
    W2 = np.asarray(inputs["W2"], dtype=np.float32)
    b2 = np.asarray(inputs["b2"], dtype=np.float32)
    g1 = np.asarray(inputs["g1"], dtype=np.float32)
    be1 = np.asarray(inputs["be1"], dtype=np.float32)
    g2 = np.asarray(inputs["g2"], dtype=np.float32)
    be2 = np.asarray(inputs["be2"], dtype=np.float32)

    bf = ml_dtypes.bfloat16
    # [C, H] -> [P, NCT, H] (partition-major contraction tiles)
    def wtile(w):
        return np.ascontiguousarray(w.reshape(NCT, P, H).transpose(1, 0, 2))

    wq_f = _to_f8(wtile(SW * (g1[:, None] * Wq)))
    wk_f = _to_f8(wtile(SW * (g1[:, None] * Wk)))
    wv_f = _to_f8(wtile(SW * (g1[:, None] * Wv)))
    qb = (SW * (be1 @ Wq)).reshape(NH, P).T
    kb = (SW * (be1 @ Wk)).reshape(NH, P).T
    vb = be1 @ Wv
    w1_full = SW * (g2[:, None] * W1)
    w1_f = _to_f8(w1_full.reshape(NCT, P, NF, P).transpose(1, 2, 0, 3))
    w2_f = _to_f8((SW * W2).reshape(NF, P, C).transpose(1, 0, 2))
    b1_f = (SW * (b1 + be2 @ W1)).reshape(NF, P).T
    cb = np.ascontiguousarray(
        np.concatenate([qb, kb, b1_f], axis=1).astype(np.float32)
    )

    # per-half masks over the PERMUTED key space: for own block lp the only
    # partially/fully masked tiles sit in its last 512-key chunk (the
    # permutations are constructed to guarantee this).
    qr = np.arange(P)[:, None]
    rk = np.arange(P)[None, :]
    masks_h = []
    for half in range(2):
        PERM, L = PERMS[half], L_HALF[half]
        mk = np.empty((NLOC, P, 512), dtype=bf)
        for lp in range(NLOC):
            nch, qg = NCHUNKS[lp], L[lp]
            for tt, g in enumerate(PERM[4 * (nch - 1) : 4 * nch]):
                if g < qg:
                    m = np.zeros((P, P), dtype=np.float32)
                elif g > qg:
                    m = np.full((P, P), NEG, dtype=np.float32)
                else:
                    m = np.where(rk <= qr, 0.0, NEG)
                mk[lp][:, tt * P : (tt + 1) * P] = m.astype(bf)
        masks_h.append(mk)

    shared = {
        "wq": wq_f, "wk": wk_f, "wv": wv_f, "w1": w1_f, "w2": w2_f,
        "cb": cb, "vb": vb, "b2": b2,
    }
    in_maps = []
    for core in range(8):
        b, half = core // 2, core % 2
        L = L_HALF[half]
        rows = np.concatenate([np.arange(i * P, (i + 1) * P) for i in L])
        prows = np.concatenate(
            [np.arange(g * P, (g + 1) * P) for g in PERMS[half]]
        )
        m = dict(shared)
        m["x"] = np.ascontiguousarray(x[b][prows])  # permuted key space
        m["x_own"] = np.ascontiguousarray(x[b][rows])
        m["masks"] = masks_h[half]
        in_maps.append(m)
    return in_maps


def _scatter_out(results):
    out = np.empty((B, T, C), dtype=np.float32)
    for core in range(8):
        b, half = core // 2, core % 2
        L = L_HALF[half]
        o = results[core]["out"]
        for ppos, i in enumerate(L):
            out[b, i * P : (i + 1) * P, :] = o[ppos * P : (ppos + 1) * P, :]
    return out


def run(inputs, trace=False, **kw):
    nc = _get_nc()
    in_maps = _prep_host(inputs)
    res = run_bass_kernel_spmd(
        nc, in_maps, core_ids=list(range(8)), trace=trace, **kw
    )
    return _scatter_out(res.results), res


def kernel(**inputs) -> np.ndarray:
    out, _ = run(inputs, trace=False)
    return out
